# revision 1
# baseline (speedup 1.0000x reference)
"""Trainium2 Bass kernel for CRF logZ (nn_CRFModel).

Math: probability-space forward recurrence with a constant per-step rescale
folded into the transitions (expAs = exp(A - log64)); the state
p~ = exp(alpha - t*log64) stays in ~[1e-5, 1e-1] so no per-step
normalization is needed.  logZ = log(expAs[:,EOS]^T p~_T) + 129*log64.

Per core (data-parallel, 32 sentences each):
  1. xbar dma_gather(transpose=True) pulls the 4096 needed E rows (fp16)
     from two half-vocab tables (int16 index limit) directly in
     D-on-partitions layout: out[p, c, w] = E[word_w, 128c+p].
  2. copy_predicated merges the two gathers (hi-vocab words overwrite).
  3. GEMM emis[tag, w] = ThetaB @ Erows^T in fp16, N=512 per matmul.
  4. exp on ScalarE -> expE.
  5. 128-step recurrence split into two 16-sentence chains, phase-
     interleaved so PE/DVE semaphore latency of one chain hides under the
     other's work: q = expAs^T p (PE, fp16), p' = q * expE_t (DVE).
Masking: expAs[:, BOS]=0, expAs[EOS, :]=0, and the final contraction
column has EOS entry 0 - exactly equivalent to the reference's NEG masks.
"""

import sys

for _p in ("/opt/trn_rl_repo", "/root/.axon_site/_ro/trn_rl_repo"):
    if _p not in sys.path:
        sys.path.insert(0, _p)

import math

import numpy as np

import concourse.bass as bass
import concourse.mybir as mybir
import concourse.tile as tile
from concourse import bacc
from concourse.bass_utils import run_bass_kernel_spmd
from concourse.tile import add_dep_helper


K = 64
V = 50257
D = 512
BT = 256
T = 128
BOS = 62
EOS = 63
N_CORES = 8
B_PER_CORE = BT // N_CORES          # 32 sentences per core
HB = B_PER_CORE // 2                # 16 sentences per chain
W_PER_CORE = B_PER_CORE * T         # 4096 gathered words per core
VSPLIT = 32768                      # int16 index limit
NW_G = 512                          # max words per gather instruction
# words per gather group (tried [128,384]+[512]*7 to shrink the head: the
# first mul starts 11us sooner but the extra group boundaries stretch the
# PE-saturated recurrence by the same amount - uniform is best measured).
GROUPS = [512] * 8
assert sum(GROUPS) == W_PER_CORE
N_G = len(GROUPS)
LOG64 = math.log(64.0)

F32 = mybir.dt.float32
F16 = mybir.dt.float16
I16 = mybir.dt.int16
U8 = mybir.dt.uint8

_CACHE = {}


def _build():
    nc = bacc.Bacc("TRN2", target_bir_lowering=False, debug=False,
                   num_devices=N_CORES)

    S = W_PER_CORE // 16  # 256 idx slots per partition-row
    idx_d = nc.dram_tensor("idx2", [128, 2 * S], I16, kind="ExternalInput").ap()
    msk_d = nc.dram_tensor("maskhi", [128, 4 * W_PER_CORE], U8,
                           kind="ExternalInput").ap()
    wa_d = nc.dram_tensor("WA", [K, K], F32, kind="ExternalInput").ap()
    amask_d = nc.dram_tensor("amask", [K, K], F32, kind="ExternalInput").ap()
    thT_d = nc.dram_tensor("ThetaBT", [4, 128, K], F16,
                           kind="ExternalInput").ap()
    p0_d = nc.dram_tensor("p0", [K, HB], F16, kind="ExternalInput").ap()
    elo_d = nc.dram_tensor("Elo", [VSPLIT, D], F16, kind="ExternalInput").ap()
    ehi_d = nc.dram_tensor("Ehi", [V - VSPLIT, D], F16,
                           kind="ExternalInput").ap()
    out_d = nc.dram_tensor("out", [1, B_PER_CORE], F32,
                           kind="ExternalOutput").ap()

    with tile.TileContext(nc) as tc:
        with (
            tc.tile_pool(name="const", bufs=1) as cpool,
            tc.tile_pool(name="gat", bufs=8) as gpool,
            tc.tile_pool(name="pst", bufs=3) as ppool,
            tc.tile_pool(name="psum_em", bufs=2, space="PSUM") as ps_em,
            tc.tile_pool(name="psum_qa", bufs=3, space="PSUM") as ps_qa,
            tc.tile_pool(name="psum_qb", bufs=3, space="PSUM") as ps_qb,
        ):
            # ---- constants ------------------------------------------------
            # one combined idx DMA first: the gathers gate on nothing else
            idx2 = cpool.tile([128, 2 * S], I16, tag="idx2")
            nc.gpsimd.dma_start(idx2[:], idx_d[:])
            ilo = idx2[:, 0:S]
            ihi = idx2[:, S:2 * S]

            msks = []
            moff = 0
            for g, nw in enumerate(GROUPS):
                m_g = cpool.tile([128, 4 * nw], U8, tag=f"msk{g}")
                nc.sync.dma_start(m_g[:], msk_d[:, moff:moff + 4 * nw])
                msks.append(m_g)
                moff += 4 * nw

            wa_sb = cpool.tile([K, K], F32, tag="wa")
            nc.sync.dma_start(wa_sb[:], wa_d[:])
            amask = cpool.tile([K, K], F32, tag="amask")
            nc.sync.dma_start(amask[:], amask_d[:])

            # expAs = exp(WA - log64) * mask (mask: col BOS = 0, row EOS = 0)
            nlog64 = cpool.tile([K, 1], F32, tag="nlog64")
            nc.vector.memset(nlog64[:], -LOG64)
            expas = cpool.tile([K, K], F32, tag="expas")
            nc.scalar.activation(expas[:], wa_sb[:],
                                 mybir.ActivationFunctionType.Exp,
                                 bias=nlog64[:], scale=1.0)
            expas_bf = cpool.tile([K, K], F16, tag="expas_bf")
            nc.vector.tensor_mul(expas_bf[:], expas[:], amask[:])

            # ThetaB^T fp16 chunks [128, 64] (host pre-transposed)
            thT = []
            for c in range(4):
                t_bf = cpool.tile([128, K], F16, tag=f"thT{c}")
                nc.sync.dma_start(t_bf[:], thT_d[c])
                thT.append(t_bf)

            # initial state p0 = one-hot(BOS), two half-batch chains
            pA = ppool.tile([K, HB], F16, tag="pA")
            nc.sync.dma_start(pA[:], p0_d[:])
            pB = ppool.tile([K, HB], F16, tag="pB")
            nc.sync.dma_start(pB[:], p0_d[:])

            # ---- pipeline over 8 groups of 512 words (16 steps each) ------
            # Order-only anchors so the scheduler interleaves each group's
            # emission work into the previous group's recurrence instead of
            # running the whole emission phase first (PE/DVE are FIFO).
            rec_mm = []   # recurrence matmul instructions of previous group
            rec_mul = []  # recurrence multiply instructions of previous group
            woff = 0
            for g, nw in enumerate(GROUPS):
                sl = slice(woff // 16, (woff + nw) // 16)
                glo = gpool.tile([128, 4 * nw], F16, tag="glo")
                nc.gpsimd.dma_gather(
                    glo[:].rearrange("p (c w) -> p c w", c=4),
                    elo_d[:], ilo[:, sl], nw, nw, D, transpose=True)
                ghi = gpool.tile([128, 4 * nw], F16, tag="ghi")
                nc.gpsimd.dma_gather(
                    ghi[:].rearrange("p (c w) -> p c w", c=4),
                    ehi_d[:], ihi[:, sl], nw, nw, D, transpose=True)
                mrg = nc.vector.copy_predicated(glo[:], msks[g][:], ghi[:])
                if rec_mul:
                    add_dep_helper(mrg.ins, rec_mul[len(rec_mul) // 4].ins,
                                   reason="interleave merge into prev recurrence")

                em_ps = ps_em.tile([K, nw], F32, tag="em")
                for c in range(4):
                    mm = nc.tensor.matmul(em_ps[:], lhsT=thT[c][:],
                                          rhs=glo[:, c * nw:(c + 1) * nw],
                                          start=(c == 0), stop=(c == 3))
                    if rec_mm and c == 0:
                        add_dep_helper(mm.ins, rec_mm[(len(rec_mm) * 5) // 8].ins,
                                       reason="interleave gemm into prev recurrence")
                expe = cpool.tile([K, nw], F32, tag=f"expe{g}")
                nc.scalar.activation(expe[:], em_ps[:],
                                     mybir.ActivationFunctionType.Exp)

                rec_mm, rec_mul = [], []
                for tt in range(nw // B_PER_CORE):
                    w0 = tt * B_PER_CORE
                    qa = ps_qa.tile([K, HB], F32, tag="qa")
                    rec_mm.append(
                        nc.tensor.matmul(qa[:], lhsT=expas_bf[:], rhs=pA[:],
                                         start=True, stop=True))
                    qb = ps_qb.tile([K, HB], F32, tag="qb")
                    rec_mm.append(
                        nc.tensor.matmul(qb[:], lhsT=expas_bf[:], rhs=pB[:],
                                         start=True, stop=True))
                    pA = ppool.tile([K, HB], F16, tag="pA")
                    rec_mul.append(
                        nc.vector.tensor_mul(pA[:], qa[:],
                                             expe[:, w0:w0 + HB]))
                    pB = ppool.tile([K, HB], F16, tag="pB")
                    rec_mul.append(
                        nc.vector.tensor_mul(pB[:], qb[:],
                                             expe[:, w0 + HB:w0 + B_PER_CORE]))
                woff += nw

            # ---- finale ---------------------------------------------------
            z = ps_em.tile([1, B_PER_CORE], F32, tag="em")
            nc.tensor.matmul(z[:, 0:HB], lhsT=expas_bf[:, EOS:EOS + 1],
                             rhs=pA[:], start=True, stop=True)
            nc.tensor.matmul(z[:, HB:B_PER_CORE],
                             lhsT=expas_bf[:, EOS:EOS + 1],
                             rhs=pB[:], start=True, stop=True)
            lnz = cpool.tile([1, B_PER_CORE], F32, tag="lnz")
            nc.scalar.activation(lnz[:], z[:], mybir.ActivationFunctionType.Ln)
            res = cpool.tile([1, B_PER_CORE], F32, tag="res")
            nc.vector.tensor_scalar_add(res[:], lnz[:], float((T + 1) * LOG64))
            nc.sync.dma_start(out_d[:], res[:])

    nc.compile()
    return nc


def _get_nc():
    if "nc" not in _CACHE:
        _CACHE["nc"] = _build()
    return _CACHE["nc"]


def _wrap16(w):
    """idx j -> partition j%16, slot j//16; replicated to all 8 Q7 cores."""
    a = np.asarray(w, np.int16).reshape(-1, 16).T  # [16, S]
    return np.tile(a, (8, 1))                      # [128, S]


def _make_in_maps(words, WA, ThetaB, E):
    words = np.asarray(words)
    WA = np.ascontiguousarray(np.asarray(WA, np.float32))
    ThetaB = np.asarray(ThetaB, np.float32)
    E = np.asarray(E, np.float32)
    Elo = np.ascontiguousarray(E[:VSPLIT].astype(np.float16))
    Ehi = np.ascontiguousarray(E[VSPLIT:].astype(np.float16))
    # ThetaB^T [512, 64] -> [4, 128, 64] fp16 chunks
    ThT = np.ascontiguousarray(
        ThetaB.T.reshape(4, 128, K).astype(np.float16))
    amask = np.ones((K, K), np.float32)
    amask[:, BOS] = 0.0
    amask[EOS, :] = 0.0
    p0 = np.zeros((K, HB), np.float16)
    p0[BOS, :] = 1.0

    in_maps = []
    for c in range(N_CORES):
        wb = words[c * B_PER_CORE:(c + 1) * B_PER_CORE].astype(np.int64)
        wf = wb.T.reshape(-1)                    # t-major flat: j = t*32 + b
        is_hi = wf >= VSPLIT
        wlo = np.where(is_hi, 0, wf).astype(np.int16)
        whi = np.where(is_hi, wf - VSPLIT, 0).astype(np.int16)
        parts, off = [], 0
        for nw in GROUPS:
            parts.append(np.tile(is_hi[off:off + nw], 4))
            off += nw
        m = np.concatenate(parts)
        mask = np.repeat(m.astype(np.uint8)[None, :], 128, axis=0)
        in_maps.append({
            "idx2": np.ascontiguousarray(
                np.concatenate([_wrap16(wlo), _wrap16(whi)], axis=1)),
            "maskhi": np.ascontiguousarray(mask),
            "WA": WA, "amask": amask, "ThetaBT": ThT, "p0": p0,
            "Elo": Elo, "Ehi": Ehi,
        })
    return in_maps


def kernel(words, WA, ThetaB, E):
    nc = _get_nc()
    in_maps = _make_in_maps(words, WA, ThetaB, E)
    res = run_bass_kernel_spmd(nc, in_maps, list(range(N_CORES)))
    return np.concatenate(
        [res.results[c]["out"][0] for c in range(N_CORES)]).astype(np.float32)



# revision 5
# speedup vs baseline: 1.3255x; 1.3255x over previous
"""Trainium2 Bass kernel for CRF logZ (nn_CRFModel).

Math: probability-space recurrence with the per-step 1/64 rescale folded
into As = exp(WA - log64) (masked: col BOS = 0, row EOS = 0).

    logZ = ln(a^T prod_t(D_t As^T) p0) + 129*log64,  D_t = diag(exp(emis_t))

The product is evaluated from BOTH ends meeting at t=64: a forward vector
chain p and a backward vector chain gamma run concurrently, fused into a
SINGLE PE matmul per round via the block-diagonal stationary
W = [[As, 0], [0, As^T]] acting on the stacked state X = [p; gamma]
([128, 32]).  63 fused rounds of (PE matmul -> DVE multiply) replace the
baseline's 128, and each round is one matmul + one [128,32] multiply
instead of two of each.

Emissions: a single fp8(e4m3) copy of E is gathered with
dma_gather(transpose=True) using SIGNED int16 indices based at row 32768
(the Q7 descriptor math is base + stride*signed_idx), so one gather per
word covers the whole 50257-row vocab - no lo/hi double gather, no merge
masks.  Each gather group is padded to a multiple of 128 indices with
idx 0 (>= 0) because the Q7 kernel trims trailing NEGATIVE indices.
Forward-pair emissions GEMM into PSUM partitions 0:64, backward into
64:128, one shared exp per pair; each round's DVE multiply then reads one
[128, 32] column block covering both chains.
"""

import sys

for _p in ("/opt/trn_rl_repo", "/root/.axon_site/_ro/trn_rl_repo"):
    if _p not in sys.path:
        sys.path.insert(0, _p)

import math

import ml_dtypes
import numpy as np

import concourse.bass as bass
import concourse.mybir as mybir
import concourse.tile as tile
from concourse import bacc
from concourse.bass_utils import run_bass_kernel_spmd
from concourse.tile import add_dep_helper

K = 64
V = 50257
D = 512
BT = 256
T = 128
BOS = 62
EOS = 63
N_CORES = 8
B = BT // N_CORES                   # 32 sentences per core
VOFF = 32768                        # signed-idx base row of the E table
LOG64 = math.log(64.0)

# pair structure: forward t-ranges ascending, backward descending, meeting
# at t=64.  Round r (0..62): fwd multiplies by e_{r+1}, bwd by e_{126-r}.
FWD_T = [(1, 4), (5, 16), (17, 32), (33, 48), (49, 63)]
BWD_T = [(126, 123), (122, 111), (110, 95), (94, 79), (78, 64)]
NW = [128, 384, 512, 512, 480]      # real words per pair per direction
NWPAD = [256, 512, 640, 640, 512]   # padded to %128 with a >=0 pad block
ROUND0 = [0, 4, 16, 32, 48]         # first round of each pair
N_ROUNDS = 63
# rounds (within the previous pair's span) after which pair p's q-th GEMM
# matmul is anchored, so emission work interleaves into the recurrence
GEMM_ANCHOR = {
    1: [0, 0, 1, 1, 2, 2, 3, 3],
    2: [4, 5, 7, 8, 10, 11, 13, 14],
    3: [16, 18, 20, 22, 24, 26, 28, 30],
    4: [32, 34, 36, 38, 40, 42, 44, 46],
}

F32 = mybir.dt.float32
F16 = mybir.dt.float16
F8 = mybir.dt.float8e4
I16 = mybir.dt.int16

S_TOTAL = (128 + 2 * sum(NWPAD)) // 16   # idx columns (16-wrapped)

_CACHE = {}


def _build():
    nc = bacc.Bacc("TRN2", target_bir_lowering=False, debug=False,
                   num_devices=N_CORES)

    idx_d = nc.dram_tensor("idx", [128, S_TOTAL], I16,
                           kind="ExternalInput").ap()
    tht_d = nc.dram_tensor("thT", [128, 4 * K], F16,
                           kind="ExternalInput").ap()
    w_d = nc.dram_tensor("Wbd", [128, 128], F16, kind="ExternalInput").ap()
    ci_d = nc.dram_tensor("cinit", [128, B], F32, kind="ExternalInput").ap()
    e8_d = nc.dram_tensor("E8", [V, D], F8, kind="ExternalInput").ap()
    out_d = nc.dram_tensor("out", [1, B], F32, kind="ExternalOutput").ap()

    with tile.TileContext(nc) as tc:
        with (
            tc.tile_pool(name="const", bufs=1) as cpool,
            tc.tile_pool(name="x", bufs=3) as xpool,
            tc.tile_pool(name="ps_em", bufs=2, space="PSUM") as ps_em,
            tc.tile_pool(name="ps_y", bufs=3, space="PSUM") as ps_y,
            tc.tile_pool(name="ps_z", bufs=1, space="PSUM") as ps_z,
        ):
            # ---- constants -----------------------------------------------
            idx = cpool.tile([128, S_TOTAL], I16, tag="idx")
            nc.gpsimd.dma_start(idx[:], idx_d[:])
            tht = cpool.tile([128, 4 * K], F16, tag="tht")
            nc.sync.dma_start(tht[:], tht_d[:])
            wsb = cpool.tile([128, 128], F16, tag="wsb")
            nc.sync.dma_start(wsb[:], w_d[:])
            cin = cpool.tile([128, B], F32, tag="cin")
            nc.sync.dma_start(cin[:], ci_d[:])
            ones = cpool.tile([128, 1], F16, tag="ones")
            nc.vector.memset(ones[:], 1.0)

            ebase = e8_d[VOFF:VOFF + 2]

            # ---- all gathers up front (DMA pipelines while PE works) -----
            col = 0
            gt = []                       # gather tiles: [mini, f0, b0, ...]
            for gi, nwp in enumerate([128] + [n for p in NWPAD
                                              for n in (p, p)]):
                g = cpool.tile([128, 4 * nwp], F8, tag=f"g{gi}")
                nc.gpsimd.dma_gather(
                    g[:].rearrange("p (c w) -> p c w", c=4),
                    ebase, idx[:, col // 16:(col + nwp) // 16],
                    nwp, nwp, D, transpose=True)
                gt.append(g)
                col += nwp

            def gemm4(em_half, g, w0, w1):
                """4 accumulating matmuls: emis over gathered fp8 words
                [w0:w1), 16-bit-interleaved layout (c16, w, byte)."""
                v = g[:].rearrange("p (c w j) -> p c w j", c=2, j=2)
                mms = []
                for q in range(4):
                    c16, jj = q // 2, q % 2
                    mms.append(nc.tensor.matmul(
                        em_half[:, 0:w1 - w0],
                        lhsT=tht[:, K * q:K * (q + 1)],
                        rhs=v[:, c16, w0:w1, jj],
                        start=(q == 0), stop=(q == 3)))
                return mms

            # ---- mini pair: fwd t=0 words (cols 0:32), bwd t=127 (32:64) -
            em0 = ps_em.tile([128, 512], F32, tag="em")
            gemm4(em0[0:64, :], gt[0], 0, B)
            gemm4(em0[64:128, :], gt[0], B, 2 * B)
            ee0 = cpool.tile([128, B], F32, tag="ee_mini")
            nc.scalar.activation(ee0[:], em0[:, 0:B],
                                 mybir.ActivationFunctionType.Exp)
            x = xpool.tile([128, B], F16, tag="x")
            nc.vector.tensor_mul(x[:], cin[:], ee0[:])   # X0 = [p_1; g_127]

            # ---- pair 0 emission (needed from round 0) -------------------
            expe = [None] * 5
            em_p = ps_em.tile([128, 512], F32, tag="em")
            gemm4(em_p[0:64, :], gt[1], 0, NW[0])
            gemm4(em_p[64:128, :], gt[2], 0, NW[0])
            expe[0] = cpool.tile([128, NW[0]], F32, tag="ee0", name="ee0")
            nc.scalar.activation(expe[0][:], em_p[:, 0:NW[0]],
                                 mybir.ActivationFunctionType.Exp)

            # lazy emission state for pairs 1..4
            emit_plan = {}                   # round -> list of (pair, q)
            for p, rounds in GEMM_ANCHOR.items():
                for q, r in enumerate(rounds):
                    emit_plan.setdefault(r, []).append((p, q))
            em_tiles = {0: em_p}

            # ---- 63 fused rounds ----------------------------------------
            pair = 0
            for r in range(N_ROUNDS):
                if pair + 1 < 5 and r == ROUND0[pair + 1]:
                    pair += 1
                k = r - ROUND0[pair]

                y = ps_y.tile([128, B], F32, tag="y")
                mm = nc.tensor.matmul(y[:], lhsT=wsb[:], rhs=x[:],
                                      start=True, stop=True)
                x = xpool.tile([128, B], F16, tag="x")
                nc.vector.tensor_mul(x[:], y[:],
                                     expe[pair][:, B * k:B * (k + 1)])

                # interleave the next pair's GEMM into this pair's rounds
                for (p, q) in emit_plan.get(r, ()):
                    if q == 0:
                        em_tiles[p] = ps_em.tile([128, 512], F32, tag="em",
                                                 name=f"em{p}")
                    emh = em_tiles[p]
                    half = emh[0:64, :] if q < 4 else emh[64:128, :]
                    g = gt[1 + 2 * p + (0 if q < 4 else 1)]
                    qq = q % 4
                    c16, jj = qq // 2, qq % 2
                    v = g[:].rearrange("p (c w j) -> p c w j", c=2, j=2)
                    gm = nc.tensor.matmul(
                        half[:, 0:NW[p]],
                        lhsT=tht[:, K * qq:K * (qq + 1)],
                        rhs=v[:, c16, 0:NW[p], jj],
                        start=(qq == 0), stop=(qq == 3))
                    add_dep_helper(gm.ins, mm.ins,
                                   reason="interleave gemm into recurrence")
                    if q == 7:
                        expe[p] = cpool.tile([128, NW[p]], F32, tag=f"ee{p}",
                                             name=f"ee{p}")
                        nc.scalar.activation(
                            expe[p][:], emh[:, 0:NW[p]],
                            mybir.ActivationFunctionType.Exp)

            # ---- finale: Z~ = gamma64^T As^T p64 -------------------------
            yf = ps_y.tile([128, B], F32, tag="y")
            nc.tensor.matmul(yf[64:128, :], lhsT=wsb[0:64, 0:64],
                             rhs=x[0:64, :], start=True, stop=True)
            yfs = cpool.tile([128, B], F16, tag="yfs")
            nc.scalar.activation(yfs[64:128, :], yf[64:128, :],
                                 mybir.ActivationFunctionType.Copy,
                                 scale=64.0)
            z1 = cpool.tile([128, B], F16, tag="z1")
            nc.vector.tensor_mul(z1[64:128, :], yfs[64:128, :], x[64:128, :])
            z2 = ps_z.tile([1, B], F32, tag="z")
            nc.tensor.matmul(z2[:], lhsT=ones[64:128, :], rhs=z1[64:128, :],
                             start=True, stop=True)
            res = cpool.tile([1, B], F32, tag="res")
            nc.scalar.copy(res[:], z2[:])
            nc.sync.dma_start(out_d[:], res[:])

    nc.compile()
    return nc


def _get_nc():
    if "nc" not in _CACHE:
        _CACHE["nc"] = _build()
    return _CACHE["nc"]


def _wrap16(vals):
    """slot j -> partition j%16, col j//16; replicated to all 8 Q7 cores."""
    a = np.asarray(vals, np.int16).reshape(-1, 16).T
    return np.tile(a, (8, 1))


def _make_in_maps(words, WA, ThetaB, E):
    words = np.asarray(words)
    WA = np.asarray(WA, np.float32)
    ThetaB = np.asarray(ThetaB, np.float32)
    E = np.asarray(E, np.float32)

    As = np.exp(WA - LOG64)
    As[:, BOS] = 0.0
    As[EOS, :] = 0.0
    W = np.zeros((128, 128), np.float16)
    W[:64, :64] = As
    W[64:, 64:] = As.T
    cin = np.empty((128, B), np.float32)
    cin[:64, :] = As[BOS, :][:, None]      # p_1 = e_0 * As[BOS, :]
    cin[64:, :] = As[:, EOS][:, None]      # gamma_127 = e_127 * As[:, EOS]

    # ThetaB^T in the gather's 16-bit-interleaved layout:
    # chunk q=(2*c16+j): thT[p, 64q+tag] = ThetaB[tag, 256*c16 + 2p + j]
    tht = np.empty((128, 4 * K), np.float16)
    p_ar = np.arange(128)
    for q in range(4):
        c16, j = q // 2, q % 2
        tht[:, K * q:K * (q + 1)] = ThetaB[:, 256 * c16 + 2 * p_ar + j].T

    E8 = np.ascontiguousarray(E.astype(ml_dtypes.float8_e4m3fn))

    in_maps = []
    for c in range(N_CORES):
        wb = words[c * B:(c + 1) * B].astype(np.int64)   # [32, 128]

        def block(ts, pad_to):
            w = wb[:, ts].T.reshape(-1)
            iv = (w - VOFF).astype(np.int16)
            out = np.zeros(pad_to, np.int16)             # pad idx 0 (>= 0)
            out[:len(iv)] = iv
            return out

        parts = [block([0], 64)[:32], block([127], 64)[:32],
                 np.zeros(64, np.int16)]                 # mini: 128 slots
        for p in range(5):
            parts.append(block(range(FWD_T[p][0], FWD_T[p][1] + 1),
                               NWPAD[p]))
            parts.append(block(range(BWD_T[p][0], BWD_T[p][1] - 1, -1),
                               NWPAD[p]))
        idx = np.hstack([_wrap16(np.concatenate(parts[:3]))]
                        + [_wrap16(b) for b in parts[3:]])
        in_maps.append({
            "idx": np.ascontiguousarray(idx),
            "thT": tht, "Wbd": W, "cinit": cin, "E8": E8,
        })
    return in_maps


def kernel(words, WA, ThetaB, E):
    nc = _get_nc()
    in_maps = _make_in_maps(words, WA, ThetaB, E)
    res = run_bass_kernel_spmd(nc, in_maps, list(range(N_CORES)))
    z = np.concatenate([res.results[c]["out"][0] for c in range(N_CORES)])
    return (np.log(z.astype(np.float64)) + 128 * LOG64).astype(np.float32)


# revision 6
# speedup vs baseline: 2.0051x; 1.5127x over previous
"""Trainium2 Bass kernel for CRF logZ (nn_CRFModel).

Math: probability-space recurrence with the per-step 1/64 rescale folded
into As = exp(WA - log64) (masked: col BOS = 0, row EOS = 0).

    logZ = ln(a^T prod_t(D_t As^T) p0) + 129*log64,  D_t = diag(exp(emis_t))

The product is evaluated from BOTH ends meeting at t=64: a forward vector
chain p and a backward vector chain gamma run concurrently, fused into a
SINGLE PE matmul per round via the block-diagonal stationary
W = [[As, 0], [0, As^T]] acting on the stacked state X = [p; gamma]
([128, 32]).  63 fused rounds of (PE matmul -> DVE multiply) replace a
naive 128, each round one matmul + one [128,32] multiply.

Emissions: a single fp8(e4m3) copy of E is gathered with
dma_gather(transpose=True) using SIGNED int16 indices based at row 32768
(the Q7 descriptor math is base + stride*signed_idx), so one gather per
word covers the whole 50257-row vocab.  Each pair's fwd+bwd words are
PACKED into one gather (fwd at slots [0,nw), bwd at [nw,2nw)); the GEMM
uses a [128,128] lhsT with ThetaB^T duplicated in both column halves so
one matmul per D-chunk emits fwd tags for fwd words and bwd tags for bwd
words in one pass (the off-blocks are never read).  Gathers are spread
over all 4 SWDGE queues (parallel Q7 descriptor generation), and padding
costs nothing: each group is [real words, sentinel idx 0, -1 pads] with
num_idxs_reg = real+1 — the Q7 trims trailing negative idxs, so pads
generate no descriptors while the SBUF layout keeps the padded stride.
"""

import sys

for _p in ("/opt/trn_rl_repo", "/root/.axon_site/_ro/trn_rl_repo"):
    if _p not in sys.path:
        sys.path.insert(0, _p)

import math

import ml_dtypes
import numpy as np

import concourse.bass as bass
import concourse.mybir as mybir
import concourse.tile as tile
from concourse import bacc
from concourse.bass_utils import run_bass_kernel_spmd
from concourse.tile import add_dep_helper

K = 64
V = 50257
D = 512
BT = 256
T = 128
BOS = 62
EOS = 63
N_CORES = 8
B = BT // N_CORES                   # 32 sentences per core
VOFF = 32768                        # signed-idx base row of the E table
LOG64 = math.log(64.0)

# 8 pairs; pair p: fwd t = 1+8p .. min(8+8p, 63), bwd t = 126-8p down.
# Round r (0..62): pair r//8, block r%8; fwd mult e_{1+r}, bwd e_{126-r}.
NBLK = [8, 8, 8, 8, 8, 8, 8, 7]     # 32-word blocks per direction
N_ROUNDS = 63
F32 = mybir.dt.float32
F16 = mybir.dt.float16
F8 = mybir.dt.float8e4
I16 = mybir.dt.int16


def _pad128(n):
    return -(-n // 128) * 128


NW = [32 * n for n in NBLK]                   # real words per direction
NREAL = [2 * n for n in NW]                   # packed fwd+bwd words
NSLOT = [_pad128(n + 1) for n in NREAL]       # slots incl sentinel+pads
MINI_SLOT = 128                               # 64 real + sentinel + pads
S_TOTAL = (MINI_SLOT + sum(NSLOT)) // 16      # idx columns (16-wrapped)
GQUEUE = [i % 4 for i in range(9)]            # queue per gather

_CACHE = {}


def _build():
    nc = bacc.Bacc("TRN2", target_bir_lowering=False, debug=False,
                   num_devices=N_CORES, num_swdge_queues=4)

    idx_d = nc.dram_tensor("idx", [128, S_TOTAL], I16,
                           kind="ExternalInput").ap()
    tht_d = nc.dram_tensor("thT", [128, 512], F16,
                           kind="ExternalInput").ap()
    w_d = nc.dram_tensor("Wbd", [128, 128], F16, kind="ExternalInput").ap()
    ci_d = nc.dram_tensor("cinit", [128, B], F32, kind="ExternalInput").ap()
    e8_d = nc.dram_tensor("E8", [V, D], F8, kind="ExternalInput").ap()
    out_d = nc.dram_tensor("out", [1, B], F32, kind="ExternalOutput").ap()

    with tile.TileContext(nc) as tc:
        with (
            tc.tile_pool(name="const", bufs=1) as cpool,
            tc.tile_pool(name="x", bufs=3) as xpool,
            tc.tile_pool(name="ps_em", bufs=2, space="PSUM") as ps_em,
            tc.tile_pool(name="ps_y", bufs=3, space="PSUM") as ps_y,
            tc.tile_pool(name="ps_z", bufs=1, space="PSUM") as ps_z,
        ):
            # ---- constants -----------------------------------------------
            idx = cpool.tile([128, S_TOTAL], I16, tag="idx")
            nc.gpsimd.dma_start(idx[:], idx_d[:])
            tht = cpool.tile([128, 512], F16, tag="tht")
            nc.sync.dma_start(tht[:], tht_d[:])
            wsb = cpool.tile([128, 128], F16, tag="wsb")
            nc.sync.dma_start(wsb[:], w_d[:])
            cin = cpool.tile([128, B], F32, tag="cin")
            nc.sync.dma_start(cin[:], ci_d[:])
            ones = cpool.tile([128, 1], F16, tag="ones")
            nc.vector.memset(ones[:], 1.0)

            ebase = e8_d[VOFF:VOFF + 2]

            # ---- all gathers up front, spread over the 4 SWDGE queues ----
            col = 0
            gt = []                       # gather tiles: [mini, P0..P7]
            for gi, (nslot, nreal) in enumerate(
                    [(MINI_SLOT, 64)] + list(zip(NSLOT, NREAL))):
                g = cpool.tile([128, 4 * nslot], F8, tag=f"g{gi}",
                               name=f"g{gi}")
                nc.gpsimd.dma_gather(
                    g[:].rearrange("p (c w) -> p c w", c=4),
                    ebase, idx[:, col // 16:(col + nslot) // 16],
                    nslot, nreal + 1, D, transpose=True,
                    queue_num=GQUEUE[gi])
                gt.append(g)
                col += nslot

            def gemm4(em, gi, ncols, anchors=None):
                """4 accumulating matmuls over packed words [0:ncols);
                the [128,128] lhsT emits fwd tags (0:64) + bwd (64:128)."""
                g = gt[gi]
                v = g[:].rearrange("p (c w j) -> p c w j", c=2, j=2)
                for q in range(4):
                    c16, jj = q // 2, q % 2
                    mm = nc.tensor.matmul(
                        em[:, 0:ncols],
                        lhsT=tht[:, 128 * q:128 * (q + 1)],
                        rhs=v[:, c16, 0:ncols, jj],
                        start=(q == 0), stop=(q == 3))
                    if anchors is not None:
                        add_dep_helper(mm.ins, anchors[q].ins,
                                       reason="interleave gemm")

            def exp2(em, nw, name):
                """exp both halves: fwd cols [0:nw], bwd cols [nw:2nw]."""
                ee = cpool.tile([128, nw], F32, tag=name, name=name)
                nc.scalar.activation(ee[0:64, :], em[0:64, 0:nw],
                                     mybir.ActivationFunctionType.Exp)
                nc.scalar.activation(ee[64:128, :], em[64:128, nw:2 * nw],
                                     mybir.ActivationFunctionType.Exp)
                return ee

            # ---- mini: fwd t=0 words (slots 0:32), bwd t=127 (32:64) -----
            em0 = ps_em.tile([128, 512], F32, tag="em")
            gemm4(em0, 0, 64)
            ee0 = exp2(em0, B, "ee_mini")
            x = xpool.tile([128, B], F16, tag="x")
            nc.vector.tensor_mul(x[:], cin[:], ee0[:])  # X0 = [p_1; g_127]

            # ---- pair 0 emission (needed from round 0) -------------------
            expe = [None] * 8
            em_p = ps_em.tile([128, 512], F32, tag="em")
            gemm4(em_p, 1, NREAL[0])
            expe[0] = exp2(em_p, NW[0], "ee0")

            # ---- 63 fused rounds, next pair's GEMM interleaved -----------
            round_mms = []
            for r in range(N_ROUNDS):
                pair, k = r // 8, r % 8

                y = ps_y.tile([128, B], F32, tag="y")
                mm = nc.tensor.matmul(y[:], lhsT=wsb[:], rhs=x[:],
                                      start=True, stop=True)
                round_mms.append(mm)
                x = xpool.tile([128, B], F16, tag="x")
                nc.vector.tensor_mul(x[:], y[:],
                                     expe[pair][:, B * k:B * (k + 1)])

                # pair p+1's emission, anchored into this pair's rounds
                if k == 6 and pair + 1 < 8:
                    p = pair + 1
                    em_n = ps_em.tile([128, 512], F32, tag="em",
                                      name=f"em{p}")
                    r0 = 8 * pair
                    gemm4(em_n, 1 + p, NREAL[p],
                          anchors=[round_mms[r0 + 1], round_mms[r0 + 3],
                                   round_mms[r0 + 5], round_mms[r0 + 6]])
                    expe[p] = exp2(em_n, NW[p], f"ee{p}")

            # ---- finale: Z~ = gamma64^T As^T p64 -------------------------
            yf = ps_y.tile([128, B], F32, tag="y")
            nc.tensor.matmul(yf[64:128, :], lhsT=wsb[0:64, 0:64],
                             rhs=x[0:64, :], start=True, stop=True)
            yfs = cpool.tile([128, B], F16, tag="yfs")
            nc.scalar.activation(yfs[64:128, :], yf[64:128, :],
                                 mybir.ActivationFunctionType.Copy,
                                 scale=64.0)
            z1 = cpool.tile([128, B], F16, tag="z1")
            nc.vector.tensor_mul(z1[64:128, :], yfs[64:128, :], x[64:128, :])
            z2 = ps_z.tile([1, B], F32, tag="z")
            nc.tensor.matmul(z2[:], lhsT=ones[64:128, :], rhs=z1[64:128, :],
                             start=True, stop=True)
            res = cpool.tile([1, B], F32, tag="res")
            nc.scalar.copy(res[:], z2[:])
            nc.sync.dma_start(out_d[:], res[:])

    nc.compile()
    return nc


def _get_nc():
    if "nc" not in _CACHE:
        _CACHE["nc"] = _build()
    return _CACHE["nc"]


def _wrap16(vals):
    """slot j -> partition j%16, col j//16; replicated to all 8 Q7 cores."""
    a = np.asarray(vals, np.int16).reshape(-1, 16).T
    return np.tile(a, (8, 1))


def _make_in_maps(words, WA, ThetaB, E):
    words = np.asarray(words)
    WA = np.asarray(WA, np.float32)
    ThetaB = np.asarray(ThetaB, np.float32)
    E = np.asarray(E, np.float32)

    As = np.exp(WA - LOG64)
    As[:, BOS] = 0.0
    As[EOS, :] = 0.0
    W = np.zeros((128, 128), np.float16)
    W[:64, :64] = As
    W[64:, 64:] = As.T
    cin = np.empty((128, B), np.float32)
    cin[:64, :] = As[BOS, :][:, None]      # p_1 = e_0 * As[BOS, :]
    cin[64:, :] = As[:, EOS][:, None]      # gamma_127 = e_127 * As[:, EOS]

    # ThetaB^T in the gather's 16-bit-interleaved layout, duplicated into
    # both lhsT column halves: chunk q=(2*c16+j):
    #   tht[p, 128q + m] = ThetaB[m % 64, 256*c16 + 2p + j]
    tht = np.empty((128, 512), np.float16)
    p_ar = np.arange(128)
    for q in range(4):
        c16, j = q // 2, q % 2
        blk = ThetaB[:, 256 * c16 + 2 * p_ar + j].T          # [128, 64]
        tht[:, 128 * q:128 * q + 64] = blk
        tht[:, 128 * q + 64:128 * (q + 1)] = blk
    E8 = np.ascontiguousarray(E.astype(ml_dtypes.float8_e4m3fn))

    in_maps = []
    for c in range(N_CORES):
        wb = words[c * B:(c + 1) * B].astype(np.int64)        # [32, 128]

        def block(f_ts, b_ts, pad_to):
            wf = wb[:, f_ts].T.reshape(-1)
            wbk = wb[:, b_ts].T.reshape(-1)
            iv = (np.concatenate([wf, wbk]) - VOFF).astype(np.int16)
            out = np.full(pad_to, -1, np.int16)   # trailing pads trimmed
            out[:len(iv)] = iv
            out[len(iv)] = 0                      # sentinel keeps reg count
            return out

        parts = [block([0], [127], MINI_SLOT)]
        t = 1
        for p in range(8):
            f_ts = list(range(t, t + NW[p] // B))
            b_ts = list(range(127 - t, 127 - t - NW[p] // B, -1))
            parts.append(block(f_ts, b_ts, NSLOT[p]))
            t += NW[p] // B
        idx = np.hstack([_wrap16(b) for b in parts])
        in_maps.append({
            "idx": np.ascontiguousarray(idx),
            "thT": tht, "Wbd": W, "cinit": cin, "E8": E8,
        })
    return in_maps


def kernel(words, WA, ThetaB, E):
    nc = _get_nc()
    in_maps = _make_in_maps(words, WA, ThetaB, E)
    res = run_bass_kernel_spmd(nc, in_maps, list(range(N_CORES)))
    z = np.concatenate([res.results[c]["out"][0] for c in range(N_CORES)])
    return (np.log(z.astype(np.float64)) + 128 * LOG64).astype(np.float32)
